# revision 9
# baseline (speedup 1.0000x reference)
"""APPNP GNN kernel distributed across 8 TRN2 NeuronCores.

Node-partitioned: core c owns nodes [c*12500, (c+1)*12500). Each PageRank
iteration: publish rho = norm*r (feature-major bf16) -> AllGather -> for each
source shard, load its slice of the gathered table into SBUF and ap_gather
the in-edge messages (GPSIMD), segment-sum them with strided DVE reduces
(vsegs sorted by sub-degree, one common run structure shared by all cores so
the SPMD program is identical), merge the 8 per-shard partial sums back to
node order with ap_gather, then apply the rho-space update
    rho_new = alpha*norm^2*(agg + rho) + 0.1*norm*h
The MLP front-end and final softmax run on TensorE/ScalarE/VectorE.
"""
import os
import sys

for _p in ("/opt/trn_rl_repo",):
    if _p not in sys.path and os.path.isdir(_p):
        sys.path.insert(0, _p)

from contextlib import ExitStack

import numpy as np
import ml_dtypes

from concourse import bacc, tile
import concourse.mybir as mybir
from concourse.bass_utils import run_bass_kernel_spmd
from concourse.masks import make_identity

N = 100000
E = 3200000
F = 512
CLS = 64
ALPHA = 0.9
ITERS = 10
NCORES = 8
SH = N // NCORES          # 12500
NT = (SH + 127) // 128    # 98
SHPAD = NT * 128          # 12544
GK = 2048                 # slots per ap_gather chunk
MCH = 1792                # merge chunk columns
dt = mybir.dt

_cache = {}


def _common_structure(all_degprofiles):
    """all_degprofiles[c][s] = sorted-desc sub-degree array (len = #vsegs).
    Returns per shard: common degree profile (desc) + chunk layout."""
    shards = []
    for s in range(NCORES):
        nv = max(len(all_degprofiles[c][s]) for c in range(NCORES)) + 1
        prof = np.zeros(nv, np.int64)
        for c in range(NCORES):
            p = all_degprofiles[c][s]
            prof[:len(p)] = np.maximum(prof[:len(p)], p)
        prof = np.sort(prof)[::-1]
        prof = np.maximum(prof, 1)  # the sentinel pad vseg has >= 1 zero-slot
        # chunk into GK-slot gather chunks; pieces = (rel_vseg, count, d)
        chunks = []
        i = 0
        vpos = 0
        while i < nv:
            used = 0
            pieces = []
            j = i
            while j < nv:
                d = int(prof[j])
                if used + d > GK:
                    break
                k = j
                cnt = 0
                while k < nv and prof[k] == d and used + (cnt + 1) * d <= GK:
                    cnt += 1
                    k += 1
                pieces.append((j - i, cnt, d))
                used += cnt * d
                j = k
            assert used > 0
            chunks.append((pieces, vpos, used))
            vpos += j - i
            i = j
        shards.append(dict(nv=nv, prof=prof, chunks=chunks))
    return shards


def _prepare(edge_index):
    src = edge_index[0].astype(np.int64)
    dst = edge_index[1].astype(np.int64)
    deg = np.bincount(dst, minlength=N).astype(np.float64) + 1.0
    norm = (1.0 / np.sqrt(deg)).astype(np.float32)

    order = np.argsort(dst, kind="stable")
    src_s = src[order]
    dst_s = dst[order]

    per_core = []
    for c in range(NCORES):
        lo, hi = np.searchsorted(dst_s, [c * SH, (c + 1) * SH])
        d_loc = dst_s[lo:hi] - c * SH
        s_glob = src_s[lo:hi]
        s_shard = (s_glob // SH).astype(np.int32)
        s_local = (s_glob % SH).astype(np.int32)
        e_order = np.lexsort((s_local, d_loc, s_shard))
        es_shard = s_shard[e_order]
        es_dst = d_loc[e_order]
        es_src = s_local[e_order]
        subdeg = np.zeros((NCORES, SH), np.int64)
        np.add.at(subdeg, (es_shard, es_dst), 1)
        shard_starts = np.searchsorted(es_shard, np.arange(NCORES + 1))
        shards = []
        for s in range(NCORES):
            a = shard_starts[s]
            dsub = subdeg[s]
            vs = np.nonzero(dsub)[0]
            dv = dsub[vs]
            vorder = np.argsort(-dv, kind="stable")
            vs = vs[vorder]
            dv = dv[vorder]
            seg_starts = a + (np.concatenate(([0], np.cumsum(dsub)))[:-1])[vs]
            shards.append((vs, dv, seg_starts))
        per_core.append(dict(norm=norm[c * SH:(c + 1) * SH],
                             shards=shards, es_src=es_src))

    profiles = [[per_core[c]["shards"][s][1] for s in range(NCORES)]
                for c in range(NCORES)]
    common = _common_structure(profiles)

    nvmax = max(sh["nv"] for sh in common)
    nvmax_pad = ((nvmax + 15) // 16) * 16

    # per-core data: idx blobs (wrapped int16) + merge blobs
    ZROW = SH  # rows SH..SHPAD-1 of every shard window are zero
    core_data = []
    for c in range(NCORES):
        pc = per_core[c]
        es_src = pc["es_src"]
        idx_cols_list = []
        mg_list = []
        for s in range(NCORES):
            vs, dv, seg_starts = pc["shards"][s]
            sh = common[s]
            nv, prof, chunks = sh["nv"], sh["prof"], sh["chunks"]
            ncv = len(vs)
            # slot stream for the common profile
            stream = np.full(sum(u for (_, _, u) in chunks) +
                             sum(GK - u for (_, _, u) in chunks), ZROW,
                             np.int32)
            # build per chunk
            pos_total = 0
            vseg_index_of_node = np.full(SH, nv - 1, np.int64)  # default: pad
            vseg_index_of_node[vs] = np.arange(ncv)
            for (pieces, vpos, used) in chunks:
                base = pos_total
                pos = 0
                for (rel, cnt, d) in pieces:
                    for t in range(cnt):
                        vi = vpos + rel + t       # common vseg index
                        if vi < ncv:
                            k = int(min(dv[vi], d))
                            st = seg_starts[vi]
                            stream[base + pos:base + pos + k] = \
                                es_src[st:st + k]
                        pos += d
                pos_total += GK
            idx_cols_list.append(stream.reshape(-1, 16).T.astype(np.int16))
            # merge indices: node v -> vseg cell (offset by shard base? no,
            # vseg buffer is reused per shard) in [0, nvmax_pad)
            mg = vseg_index_of_node.astype(np.int16)
            mgp = np.full(SHPAD, nv - 1, np.int16)
            mgp[:SH] = mg
            mg_list.append(mgp.reshape(-1, 16).T.astype(np.int16))
        # idx blob: [16, total_cols] -> tile to [64, .]
        idx_blob = np.concatenate(idx_cols_list, axis=1)
        idx_blob = np.tile(idx_blob, (4, 1))
        mg_blob = np.stack([np.tile(m, (4, 1)) for m in mg_list])
        core_data.append(dict(idx_blob=idx_blob, mg_blob=mg_blob,
                              norm=pc["norm"]))

    meta = dict(common=common, nvmax_pad=nvmax_pad,
                idx_cols=core_data[0]["idx_blob"].shape[1])
    return meta, core_data


def _build_program(meta):
    common = meta["common"]
    NVP = meta["nvmax_pad"]
    nc = bacc.Bacc("TRN2", target_bir_lowering=False, debug=False,
                   num_devices=NCORES)

    feats_d = nc.dram_tensor("feats", [SHPAD, F], dt.float32,
                             kind="ExternalInput").ap()
    w1_d = nc.dram_tensor("w1", [F, CLS], dt.bfloat16,
                          kind="ExternalInput").ap()
    w2_d = nc.dram_tensor("w2", [CLS, CLS], dt.bfloat16,
                          kind="ExternalInput").ap()
    b1_d = nc.dram_tensor("b1", [CLS, 1], dt.float32,
                          kind="ExternalInput").ap()
    b2_d = nc.dram_tensor("b2", [CLS, 1], dt.float32,
                          kind="ExternalInput").ap()
    n01_d = nc.dram_tensor("n01", [CLS, SHPAD], dt.float32,
                           kind="ExternalInput").ap()
    n2a_d = nc.dram_tensor("n2a", [CLS, SHPAD], dt.float32,
                           kind="ExternalInput").ap()
    rinv_d = nc.dram_tensor("rinv", [CLS, SHPAD], dt.float32,
                            kind="ExternalInput").ap()
    idx_d = nc.dram_tensor("idxs", [64, meta["idx_cols"]], dt.int16,
                           kind="ExternalInput").ap()
    mg_d = nc.dram_tensor("mgidx", [NCORES, 64, SHPAD // 16], dt.int16,
                          kind="ExternalInput").ap()
    out_d = nc.dram_tensor("out", [SHPAD, CLS], dt.float32,
                           kind="ExternalOutput").ap()

    with tile.TileContext(nc) as tc, ExitStack() as ctx:
        dram = ctx.enter_context(tc.tile_pool(name="dram", bufs=1,
                                              space="DRAM"))
        cpool = ctx.enter_context(tc.tile_pool(name="const", bufs=1))
        psum = ctx.enter_context(tc.tile_pool(name="psum", bufs=2,
                                              space="PSUM"))
        big = ctx.enter_context(tc.tile_pool(name="big", bufs=1))
        work = ctx.enter_context(tc.tile_pool(name="work", bufs=2))

        pub = dram.tile([CLS, SHPAD], dt.float32)
        table_dram = dram.tile([NCORES, CLS, SHPAD], dt.float32)
        h01n_dram = dram.tile([CLS, SHPAD], dt.bfloat16)

        ident = cpool.tile([128, 128], dt.bfloat16)
        make_identity(nc, ident[:])
        w1s = cpool.tile([128, 4, CLS], dt.bfloat16)
        nc.sync.dma_start(out=w1s[:],
                          in_=w1_d.rearrange("(a b) c -> b a c", b=128))
        w2s = cpool.tile([CLS, CLS], dt.bfloat16)
        nc.sync.dma_start(out=w2s[:], in_=w2_d[:])
        b1s = cpool.tile([CLS, 1], dt.float32)
        nc.sync.dma_start(out=b1s[:], in_=b1_d[:])
        b2s = cpool.tile([CLS, 1], dt.float32)
        nc.sync.dma_start(out=b2s[:], in_=b2_d[:])

        rho = big.tile([CLS, SHPAD], dt.bfloat16)
        vseg = big.tile([CLS, NVP], dt.float32)
        acc = big.tile([CLS, SHPAD], dt.bfloat16)

        # zero the publish pad region once (rows SH..SHPAD of every shard)
        zpad = cpool.tile([CLS, SHPAD - SH], dt.float32)
        nc.gpsimd.memset(zpad[:], 0.0)
        nc.sync.dma_start(out=pub[:, SH:], in_=zpad[:])

        # ---------------- MLP ----------------
        with tc.tile_pool(name="mlp", bufs=1) as mlp:
            h1s = mlp.tile([CLS, SHPAD], dt.bfloat16)
            for t in range(NT):
                xt = work.tile([128, F], dt.float32, tag="xt")
                nc.sync.dma_start(out=xt[:],
                                  in_=feats_d[t * 128:(t + 1) * 128, :])
                xtb = work.tile([128, F], dt.bfloat16, tag="xtb")
                nc.vector.tensor_copy(xtb[:], xt[:])
                xT = psum.tile([128, F], dt.bfloat16, tag="xT")
                for fc in range(4):
                    nc.tensor.transpose(
                        out=xT[:, fc * 128:(fc + 1) * 128],
                        in_=xtb[:, fc * 128:(fc + 1) * 128],
                        identity=ident[:])
                xTs = work.tile([128, F], dt.bfloat16, tag="xTs")
                nc.scalar.copy(xTs[:], xT[:])
                hp = psum.tile([CLS, 128], dt.float32, tag="hp")
                for fc in range(4):
                    nc.tensor.matmul(hp[:], lhsT=w1s[:, fc, :],
                                     rhs=xTs[:, fc * 128:(fc + 1) * 128],
                                     start=(fc == 0), stop=(fc == 3))
                nc.scalar.activation(h1s[:, t * 128:(t + 1) * 128], hp[:],
                                     mybir.ActivationFunctionType.Relu,
                                     bias=b1s[:])
            for ch in range(0, SHPAD, 448):
                ce = ch + 448
                h2p = psum.tile([CLS, 448], dt.float32, tag="h2p")
                nc.tensor.matmul(h2p[:], lhsT=w2s[:], rhs=h1s[:, ch:ce],
                                 start=True, stop=True)
                h2c = work.tile([CLS, 448], dt.float32, tag="h2c")
                nc.scalar.activation(h2c[:], h2p[:],
                                     mybir.ActivationFunctionType.Relu,
                                     bias=b2s[:])
                n01c = work.tile([CLS, 448], dt.float32, tag="n01c")
                nc.sync.dma_start(out=n01c[:], in_=n01_d[:, ch:ce])
                hc = work.tile([CLS, 448], dt.bfloat16, tag="hc")
                nc.vector.tensor_tensor(out=hc[:], in0=h2c[:], in1=n01c[:],
                                        op=mybir.AluOpType.mult)
                nc.sync.dma_start(out=h01n_dram[:, ch:ce], in_=hc[:])
                nc.vector.tensor_scalar_mul(rho[:, ch:ce], hc[:], 10.0)

        # ---------------- iterations ----------------
        winp = ctx.enter_context(tc.tile_pool(name="winp", bufs=1))
        for it in range(ITERS):
            for ch in range(0, SH, 896):
                ce = min(ch + 896, SH)
                pc_t = work.tile([CLS, 896], dt.float32, tag="n2c")
                nc.vector.tensor_copy(pc_t[:, :ce - ch], rho[:, ch:ce])
                nc.sync.dma_start(out=pub[:, ch:ce], in_=pc_t[:, :ce - ch])
            nc.gpsimd.collective_compute(
                "AllGather", mybir.AluOpType.bypass,
                replica_groups=[list(range(NCORES))],
                ins=[pub.opt()], outs=[table_dram.opt()])

            coff = 0
            for s in range(NCORES):
                window = winp.tile([CLS, SHPAD], dt.float32, tag="win")
                nc.sync.dma_start(out=window[:], in_=table_dram[s])
                for (pieces, vpos, used) in common[s]["chunks"]:
                    ixt = work.tile([64, GK // 16], dt.int16, tag="ixt")
                    nc.sync.dma_start(
                        out=ixt[:], in_=idx_d[:, coff:coff + GK // 16])
                    coff += GK // 16
                    g = work.tile([CLS, GK, 1], dt.float32, tag="g")
                    nc.gpsimd.ap_gather(g[:], window[:].unsqueeze(2), ixt[:],
                                        channels=64, num_elems=SHPAD, d=1,
                                        num_idxs=GK)
                    gf = g[:].squeeze(2)
                    pos = 0
                    for (rel, cnt, d) in pieces:
                        seg = gf[:, pos:pos + cnt * d]
                        nc.vector.tensor_reduce(
                            vseg[:, vpos + rel:vpos + rel + cnt],
                            seg.rearrange("p (n d) -> p n d", d=d),
                            mybir.AxisListType.X, mybir.AluOpType.add)
                        pos += cnt * d
                # merge this shard's vseg sums into acc (chunked)
                for mo in range(0, SHPAD, MCH):
                    mgt = work.tile([64, MCH // 16], dt.int16, tag="mgt")
                    nc.sync.dma_start(
                        out=mgt[:],
                        in_=mg_d[s, :, mo // 16:(mo + MCH) // 16])
                    mg_g = work.tile([CLS, MCH, 1], dt.float32, tag="mg_g")
                    nc.gpsimd.ap_gather(mg_g[:], vseg[:].unsqueeze(2),
                                        mgt[:], channels=64, num_elems=NVP,
                                        d=1, num_idxs=MCH)
                    if s == 0:
                        nc.vector.tensor_copy(acc[:, mo:mo + MCH],
                                              mg_g[:].squeeze(2))
                    else:
                        nc.vector.tensor_tensor(out=acc[:, mo:mo + MCH],
                                                in0=acc[:, mo:mo + MCH],
                                                in1=mg_g[:].squeeze(2),
                                                op=mybir.AluOpType.add)
            # update: rho = n2a * (acc + rho) + h01n
            for ch in range(0, SHPAD, 896):
                sl = slice(ch, ch + 896)
                nc.vector.tensor_tensor(out=acc[:, sl], in0=acc[:, sl],
                                        in1=rho[:, sl],
                                        op=mybir.AluOpType.add)
                n2c = work.tile([CLS, 896], dt.float32, tag="n2c")
                nc.sync.dma_start(out=n2c[:], in_=n2a_d[:, sl])
                nc.vector.tensor_tensor(out=acc[:, sl], in0=acc[:, sl],
                                        in1=n2c[:], op=mybir.AluOpType.mult)
                hcc = work.tile([CLS, 896], dt.bfloat16, tag="hcc")
                nc.sync.dma_start(out=hcc[:], in_=h01n_dram[:, sl])
                nc.vector.tensor_tensor(out=acc[:, sl], in0=acc[:, sl],
                                        in1=hcc[:], op=mybir.AluOpType.add)
                nc.vector.tensor_copy(rho[:, sl], acc[:, sl])

        # ---------------- softmax ----------------
        for ch in range(0, SHPAD, 896):
            sl = slice(ch, ch + 896)
            ric = work.tile([CLS, 896], dt.float32, tag="n2c")
            nc.sync.dma_start(out=ric[:], in_=rinv_d[:, sl])
            nc.vector.tensor_tensor(out=acc[:, sl], in0=rho[:, sl],
                                    in1=ric[:], op=mybir.AluOpType.mult)
        for t in range(NT):
            rT = psum.tile([128, CLS], dt.bfloat16, tag="rT")
            nc.tensor.transpose(out=rT[:],
                                in_=acc[:, t * 128:(t + 1) * 128],
                                identity=ident[:64, :64])
            rt = work.tile([128, CLS], dt.float32, tag="rt")
            nc.scalar.copy(rt[:], rT[:])
            mx = work.tile([128, 1], dt.float32, tag="mx")
            nc.vector.tensor_reduce(mx[:], rt[:], mybir.AxisListType.X,
                                    mybir.AluOpType.max, negate=True)
            ex = work.tile([128, CLS], dt.float32, tag="ex")
            nc.scalar.activation(ex[:], rt[:],
                                 mybir.ActivationFunctionType.Exp,
                                 bias=mx[:])
            sm = work.tile([128, 1], dt.float32, tag="sm")
            nc.vector.tensor_reduce(sm[:], ex[:], mybir.AxisListType.X,
                                    mybir.AluOpType.add)
            rc = work.tile([128, 1], dt.float32, tag="rc")
            nc.vector.reciprocal(rc[:], sm[:])
            ot = work.tile([128, CLS], dt.float32, tag="ot")
            nc.vector.tensor_scalar_mul(ot[:], ex[:], rc[:])
            nc.sync.dma_start(out=out_d[t * 128:(t + 1) * 128, :], in_=ot[:])

    nc.compile()
    return nc


def kernel(features, edge_index, W1, b1, W2, b2):
    features = np.asarray(features, np.float32)
    edge_index = np.asarray(edge_index)
    W1 = np.asarray(W1, np.float32)
    b1 = np.asarray(b1, np.float32)
    W2 = np.asarray(W2, np.float32)
    b2 = np.asarray(b2, np.float32)

    key = (edge_index.shape, int(edge_index[:, :64].sum()),
           int(edge_index[:, -64:].sum()))
    if key not in _cache:
        meta, core_data = _prepare(edge_index)
        nc = _build_program(meta)
        _cache[key] = (nc, meta, core_data)
    nc, meta, core_data = _cache[key]

    in_maps = []
    for c in range(NCORES):
        cd = core_data[c]
        feats = np.zeros((SHPAD, F), np.float32)
        feats[:SH] = features[c * SH:(c + 1) * SH]
        norm = np.zeros(SHPAD, np.float32)
        norm[:SH] = cd["norm"]
        n01 = np.repeat((0.1 * norm)[None, :], CLS, 0).astype(np.float32)
        n2a = np.repeat((ALPHA * norm * norm)[None, :], CLS, 0).astype(
            np.float32)
        rv = np.zeros(SHPAD, np.float32)
        rv[:SH] = 1.0 / cd["norm"]
        rinv = np.repeat(rv[None, :], CLS, 0).astype(np.float32)
        in_maps.append({
            "feats": feats,
            "w1": W1.astype(ml_dtypes.bfloat16),
            "w2": W2.astype(ml_dtypes.bfloat16),
            "b1": b1.reshape(CLS, 1).astype(np.float32),
            "b2": b2.reshape(CLS, 1).astype(np.float32),
            "n01": n01, "n2a": n2a, "rinv": rinv,
            "idxs": cd["idx_blob"], "mgidx": cd["mg_blob"],
        })
    res = run_bass_kernel_spmd(nc, in_maps, core_ids=list(range(NCORES)))
    out = np.empty((N, CLS), np.float32)
    for c in range(NCORES):
        out[c * SH:(c + 1) * SH] = \
            np.asarray(res.results[c]["out"])[:SH].astype(np.float32)
    return out


# revision 11
# speedup vs baseline: 1.9576x; 1.9576x over previous
"""APPNP GNN kernel distributed across 8 TRN2 NeuronCores.

Node-partitioned: core c owns nodes [c*12500, (c+1)*12500). Each PageRank
iteration: publish rho = norm*r (feature-major bf16) -> AllGather -> for each
source shard, load its slice of the gathered table into SBUF and ap_gather
the in-edge messages (GPSIMD), segment-sum them with strided DVE reduces
(vsegs sorted by sub-degree, one common run structure shared by all cores so
the SPMD program is identical), merge the 8 per-shard partial sums back to
node order with ap_gather, then apply the rho-space update
    rho_new = alpha*norm^2*(agg + rho) + 0.1*norm*h
The MLP front-end and final softmax run on TensorE/ScalarE/VectorE.
"""
import os
import sys

for _p in ("/opt/trn_rl_repo",):
    if _p not in sys.path and os.path.isdir(_p):
        sys.path.insert(0, _p)

from contextlib import ExitStack

import numpy as np
import ml_dtypes

from concourse import bacc, tile
import concourse.mybir as mybir
from concourse.bass_utils import run_bass_kernel_spmd
from concourse.masks import make_identity

N = 100000
E = 3200000
F = 512
CLS = 64
ALPHA = 0.9
ITERS = 10
NCORES = 8
SH = N // NCORES          # 12500
NT = (SH + 127) // 128    # 98
SHPAD = NT * 128          # 12544
GK = 2048                 # slots per ap_gather chunk
MCH = 896                 # merge chunk columns
dt = mybir.dt

_cache = {}


def _common_structure(all_degprofiles):
    """all_degprofiles[c][s] = sorted-desc sub-degree array (len = #vsegs).
    Returns per shard: common degree profile (desc) + chunk layout."""
    shards = []
    for s in range(NCORES):
        nv = max(len(all_degprofiles[c][s]) for c in range(NCORES)) + 1
        prof = np.zeros(nv, np.int64)
        for c in range(NCORES):
            p = all_degprofiles[c][s]
            prof[:len(p)] = np.maximum(prof[:len(p)], p)
        prof = np.sort(prof)[::-1]
        prof = np.maximum(prof, 1)  # the sentinel pad vseg has >= 1 zero-slot
        # chunk into GK-slot gather chunks; pieces = (rel_vseg, count, d)
        chunks = []
        i = 0
        vpos = 0
        while i < nv:
            used = 0
            pieces = []
            j = i
            while j < nv:
                d = int(prof[j])
                if used + d > GK:
                    break
                k = j
                cnt = 0
                while k < nv and prof[k] == d and used + (cnt + 1) * d <= GK:
                    cnt += 1
                    k += 1
                pieces.append((j - i, cnt, d))
                used += cnt * d
                j = k
            assert used > 0
            chunks.append((pieces, vpos, used))
            vpos += j - i
            i = j
        shards.append(dict(nv=nv, prof=prof, chunks=chunks))
    return shards


def _prepare(edge_index):
    src = edge_index[0].astype(np.int64)
    dst = edge_index[1].astype(np.int64)
    deg = np.bincount(dst, minlength=N).astype(np.float64) + 1.0
    norm = (1.0 / np.sqrt(deg)).astype(np.float32)

    order = np.argsort(dst, kind="stable")
    src_s = src[order]
    dst_s = dst[order]

    per_core = []
    for c in range(NCORES):
        lo, hi = np.searchsorted(dst_s, [c * SH, (c + 1) * SH])
        d_loc = dst_s[lo:hi] - c * SH
        s_glob = src_s[lo:hi]
        s_shard = (s_glob // SH).astype(np.int32)
        s_local = (s_glob % SH).astype(np.int32)
        e_order = np.lexsort((s_local, d_loc, s_shard))
        es_shard = s_shard[e_order]
        es_dst = d_loc[e_order]
        es_src = s_local[e_order]
        subdeg = np.zeros((NCORES, SH), np.int64)
        np.add.at(subdeg, (es_shard, es_dst), 1)
        shard_starts = np.searchsorted(es_shard, np.arange(NCORES + 1))
        shards = []
        for s in range(NCORES):
            a = shard_starts[s]
            dsub = subdeg[s]
            vs = np.nonzero(dsub)[0]
            dv = dsub[vs]
            vorder = np.argsort(-dv, kind="stable")
            vs = vs[vorder]
            dv = dv[vorder]
            seg_starts = a + (np.concatenate(([0], np.cumsum(dsub)))[:-1])[vs]
            shards.append((vs, dv, seg_starts))
        per_core.append(dict(norm=norm[c * SH:(c + 1) * SH],
                             shards=shards, es_src=es_src))

    profiles = [[per_core[c]["shards"][s][1] for s in range(NCORES)]
                for c in range(NCORES)]
    common = _common_structure(profiles)
    # pad chunk counts within each shard pair (2p, 2p+1)
    for p in range(NCORES // 2):
        a, b = common[2 * p], common[2 * p + 1]
        na, nb = len(a["chunks"]), len(b["chunks"])
        for sh, n_need in ((a, max(na, nb)), (b, max(na, nb))):
            while len(sh["chunks"]) < n_need:
                sh["chunks"].append(([], sh["nv"], 0))

    nvmax = max(sh["nv"] for sh in common)
    nvmax_pad = ((nvmax + 15) // 16) * 16

    # per-core data: idx blobs (wrapped int16) + merge blobs
    ZROW = SH  # rows SH..SHPAD-1 of every shard window are zero
    core_data = []
    for c in range(NCORES):
        pc = per_core[c]
        es_src = pc["es_src"]
        idx_cols_list = []
        mg_list = []
        for s in range(NCORES):
            vs, dv, seg_starts = pc["shards"][s]
            sh = common[s]
            nv, prof, chunks = sh["nv"], sh["prof"], sh["chunks"]
            ncv = len(vs)
            # slot stream for the common profile
            stream = np.full(sum(u for (_, _, u) in chunks) +
                             sum(GK - u for (_, _, u) in chunks), ZROW,
                             np.int32)
            # build per chunk
            pos_total = 0
            vseg_index_of_node = np.full(SH, nv - 1, np.int64)  # default: pad
            vseg_index_of_node[vs] = np.arange(ncv)
            for (pieces, vpos, used) in chunks:
                base = pos_total
                pos = 0
                for (rel, cnt, d) in pieces:
                    for t in range(cnt):
                        vi = vpos + rel + t       # common vseg index
                        if vi < ncv:
                            k = int(min(dv[vi], d))
                            st = seg_starts[vi]
                            stream[base + pos:base + pos + k] = \
                                es_src[st:st + k]
                        pos += d
                pos_total += GK
            idx_cols_list.append(stream.reshape(-1, 16).T.astype(np.int16))
            # merge indices: node v -> vseg cell (offset by shard base? no,
            # vseg buffer is reused per shard) in [0, nvmax_pad)
            mg = vseg_index_of_node.astype(np.int16)
            mgp = np.full(SHPAD, nv - 1, np.int16)
            mgp[:SH] = mg
            mg_list.append(mgp.reshape(-1, 16).T.astype(np.int16))
        # pair shards: idx tile rows 0-63 = shard 2p chunk, 64-127 = 2p+1
        pair_idx = []
        pair_mg = []
        for p in range(NCORES // 2):
            lo, hi = idx_cols_list[2 * p], idx_cols_list[2 * p + 1]
            ncols = max(lo.shape[1], hi.shape[1])
            lo2 = np.full((16, ncols), SH, np.int16)
            hi2 = np.full((16, ncols), SH, np.int16)
            lo2[:, :lo.shape[1]] = lo
            hi2[:, :hi.shape[1]] = hi
            pair_idx.append(np.concatenate(
                [np.tile(lo2, (4, 1)), np.tile(hi2, (4, 1))], axis=0))
            pair_mg.append(np.concatenate(
                [np.tile(mg_list[2 * p], (4, 1)),
                 np.tile(mg_list[2 * p + 1], (4, 1))], axis=0))
        idx_blob = np.concatenate(pair_idx, axis=1)
        mg_blob = np.stack(pair_mg)
        core_data.append(dict(idx_blob=idx_blob, mg_blob=mg_blob,
                              norm=pc["norm"]))

    meta = dict(common=common, nvmax_pad=nvmax_pad,
                idx_cols=core_data[0]["idx_blob"].shape[1])
    return meta, core_data


def _build_program(meta):
    common = meta["common"]
    NVP = meta["nvmax_pad"]
    nc = bacc.Bacc("TRN2", target_bir_lowering=False, debug=False,
                   num_devices=NCORES)

    feats_d = nc.dram_tensor("feats", [SHPAD, F], dt.float32,
                             kind="ExternalInput").ap()
    w1_d = nc.dram_tensor("w1", [F, CLS], dt.bfloat16,
                          kind="ExternalInput").ap()
    w2_d = nc.dram_tensor("w2", [CLS, CLS], dt.bfloat16,
                          kind="ExternalInput").ap()
    b1_d = nc.dram_tensor("b1", [CLS, 1], dt.float32,
                          kind="ExternalInput").ap()
    b2_d = nc.dram_tensor("b2", [CLS, 1], dt.float32,
                          kind="ExternalInput").ap()
    n01_d = nc.dram_tensor("n01", [CLS, SHPAD], dt.float32,
                           kind="ExternalInput").ap()
    n2a_d = nc.dram_tensor("n2a", [CLS, SHPAD], dt.float32,
                           kind="ExternalInput").ap()
    rinv_d = nc.dram_tensor("rinv", [CLS, SHPAD], dt.float32,
                            kind="ExternalInput").ap()
    idx_d = nc.dram_tensor("idxs", [128, meta["idx_cols"]], dt.int16,
                           kind="ExternalInput").ap()
    mg_d = nc.dram_tensor("mgidx", [NCORES // 2, 128, SHPAD // 16], dt.int16,
                          kind="ExternalInput").ap()
    out_d = nc.dram_tensor("out", [SHPAD, CLS], dt.float32,
                           kind="ExternalOutput").ap()

    with tile.TileContext(nc) as tc, ExitStack() as ctx:
        dram = ctx.enter_context(tc.tile_pool(name="dram", bufs=1,
                                              space="DRAM"))
        cpool = ctx.enter_context(tc.tile_pool(name="const", bufs=1))
        psum = ctx.enter_context(tc.tile_pool(name="psum", bufs=2,
                                              space="PSUM"))
        big = ctx.enter_context(tc.tile_pool(name="big", bufs=1))
        work = ctx.enter_context(tc.tile_pool(name="work", bufs=2))

        pub = dram.tile([CLS, SHPAD], dt.float32)
        table_dram = dram.tile([NCORES, CLS, SHPAD], dt.float32)
        h01n_dram = dram.tile([CLS, SHPAD], dt.bfloat16)

        ident = cpool.tile([128, 128], dt.bfloat16)
        make_identity(nc, ident[:])
        w1s = cpool.tile([128, 4, CLS], dt.bfloat16)
        nc.sync.dma_start(out=w1s[:],
                          in_=w1_d.rearrange("(a b) c -> b a c", b=128))
        w2s = cpool.tile([CLS, CLS], dt.bfloat16)
        nc.sync.dma_start(out=w2s[:], in_=w2_d[:])
        b1s = cpool.tile([CLS, 1], dt.float32)
        nc.sync.dma_start(out=b1s[:], in_=b1_d[:])
        b2s = cpool.tile([CLS, 1], dt.float32)
        nc.sync.dma_start(out=b2s[:], in_=b2_d[:])

        rho = big.tile([CLS, SHPAD], dt.bfloat16)
        vseg = big.tile([128, NVP], dt.float32)
        acc = big.tile([CLS, SHPAD], dt.bfloat16)

        # zero the publish pad region once (rows SH..SHPAD of every shard)
        zpad = cpool.tile([CLS, SHPAD - SH], dt.float32)
        nc.gpsimd.memset(zpad[:], 0.0)
        nc.sync.dma_start(out=pub[:, SH:], in_=zpad[:])

        # ---------------- MLP ----------------
        with tc.tile_pool(name="mlp", bufs=1) as mlp:
            h1s = mlp.tile([CLS, SHPAD], dt.bfloat16)
            for t in range(NT):
                xt = work.tile([128, F], dt.float32, tag="xt")
                nc.sync.dma_start(out=xt[:],
                                  in_=feats_d[t * 128:(t + 1) * 128, :])
                xtb = work.tile([128, F], dt.bfloat16, tag="xtb")
                nc.vector.tensor_copy(xtb[:], xt[:])
                xT = psum.tile([128, F], dt.bfloat16, tag="xT")
                for fc in range(4):
                    nc.tensor.transpose(
                        out=xT[:, fc * 128:(fc + 1) * 128],
                        in_=xtb[:, fc * 128:(fc + 1) * 128],
                        identity=ident[:])
                xTs = work.tile([128, F], dt.bfloat16, tag="xTs")
                nc.scalar.copy(xTs[:], xT[:])
                hp = psum.tile([CLS, 128], dt.float32, tag="hp")
                for fc in range(4):
                    nc.tensor.matmul(hp[:], lhsT=w1s[:, fc, :],
                                     rhs=xTs[:, fc * 128:(fc + 1) * 128],
                                     start=(fc == 0), stop=(fc == 3))
                nc.scalar.activation(h1s[:, t * 128:(t + 1) * 128], hp[:],
                                     mybir.ActivationFunctionType.Relu,
                                     bias=b1s[:])
            for ch in range(0, SHPAD, 448):
                ce = ch + 448
                h2p = psum.tile([CLS, 448], dt.float32, tag="h2p")
                nc.tensor.matmul(h2p[:], lhsT=w2s[:], rhs=h1s[:, ch:ce],
                                 start=True, stop=True)
                h2c = work.tile([CLS, 448], dt.float32, tag="h2c")
                nc.scalar.activation(h2c[:], h2p[:],
                                     mybir.ActivationFunctionType.Relu,
                                     bias=b2s[:])
                n01c = work.tile([CLS, 448], dt.float32, tag="n01c")
                nc.sync.dma_start(out=n01c[:], in_=n01_d[:, ch:ce])
                hc = work.tile([CLS, 448], dt.bfloat16, tag="hc")
                nc.vector.tensor_tensor(out=hc[:], in0=h2c[:], in1=n01c[:],
                                        op=mybir.AluOpType.mult)
                nc.sync.dma_start(out=h01n_dram[:, ch:ce], in_=hc[:])
                nc.vector.tensor_scalar_mul(rho[:, ch:ce], hc[:], 10.0)

        # ---------------- iterations ----------------
        winp = ctx.enter_context(tc.tile_pool(name="winp", bufs=1))
        for it in range(ITERS):
            for ch in range(0, SH, 896):
                ce = min(ch + 896, SH)
                pc_t = work.tile([CLS, 896], dt.float32, tag="n2c")
                nc.vector.tensor_copy(pc_t[:, :ce - ch], rho[:, ch:ce])
                nc.sync.dma_start(out=pub[:, ch:ce], in_=pc_t[:, :ce - ch])
            nc.gpsimd.collective_compute(
                "AllGather", mybir.AluOpType.bypass,
                replica_groups=[list(range(NCORES))],
                ins=[pub.opt()], outs=[table_dram.opt()])

            coff = 0
            for pr in range(NCORES // 2):
                window = winp.tile([128, SHPAD], dt.float32, tag="win")
                nc.sync.dma_start(out=window[:64], in_=table_dram[2 * pr])
                nc.sync.dma_start(out=window[64:], in_=table_dram[2 * pr + 1])
                ch_lo = common[2 * pr]["chunks"]
                ch_hi = common[2 * pr + 1]["chunks"]
                for ci in range(len(ch_lo)):
                    ixt = work.tile([128, GK // 16], dt.int16, tag="ixt")
                    nc.sync.dma_start(
                        out=ixt[:], in_=idx_d[:, coff:coff + GK // 16])
                    coff += GK // 16
                    g = work.tile([128, GK, 1], dt.float32, tag="g")
                    nc.gpsimd.ap_gather(g[:], window[:].unsqueeze(2), ixt[:],
                                        channels=128, num_elems=SHPAD, d=1,
                                        num_idxs=GK)
                    gf = g[:].squeeze(2)
                    for half, (pieces, vpos, used) in (
                            (0, ch_lo[ci]), (1, ch_hi[ci])):
                        pb = half * 64
                        pos = 0
                        for (rel, cnt, d) in pieces:
                            seg = gf[pb:pb + 64, pos:pos + cnt * d]
                            nc.vector.tensor_reduce(
                                vseg[pb:pb + 64,
                                     vpos + rel:vpos + rel + cnt],
                                seg.rearrange("p (n d) -> p n d", d=d),
                                mybir.AxisListType.X, mybir.AluOpType.add)
                            pos += cnt * d
                # merge both shards' vseg sums into acc (chunked)
                for mo in range(0, SHPAD, MCH):
                    mgt = work.tile([128, MCH // 16], dt.int16, tag="mgt")
                    nc.sync.dma_start(
                        out=mgt[:],
                        in_=mg_d[pr, :, mo // 16:(mo + MCH) // 16])
                    mg_g = work.tile([128, MCH, 1], dt.float32, tag="mg_g")
                    nc.gpsimd.ap_gather(mg_g[:], vseg[:].unsqueeze(2),
                                        mgt[:], channels=128, num_elems=NVP,
                                        d=1, num_idxs=MCH)
                    ms = work.tile([CLS, MCH], dt.float32, tag="n2c")
                    nc.sync.dma_start(out=ms[:],
                                      in_=mg_g[64:, :, :].squeeze(2))
                    if pr == 0:
                        nc.vector.tensor_copy(acc[:, mo:mo + MCH],
                                              mg_g[:64].squeeze(2))
                    else:
                        nc.vector.tensor_tensor(out=acc[:, mo:mo + MCH],
                                                in0=acc[:, mo:mo + MCH],
                                                in1=mg_g[:64].squeeze(2),
                                                op=mybir.AluOpType.add)
                    nc.vector.tensor_tensor(out=acc[:, mo:mo + MCH],
                                            in0=acc[:, mo:mo + MCH],
                                            in1=ms[:],
                                            op=mybir.AluOpType.add)
            # update: rho = n2a * (acc + rho) + h01n
            for ch in range(0, SHPAD, 896):
                sl = slice(ch, ch + 896)
                nc.vector.tensor_tensor(out=acc[:, sl], in0=acc[:, sl],
                                        in1=rho[:, sl],
                                        op=mybir.AluOpType.add)
                n2c = work.tile([CLS, 896], dt.float32, tag="n2c")
                nc.sync.dma_start(out=n2c[:], in_=n2a_d[:, sl])
                nc.vector.tensor_tensor(out=acc[:, sl], in0=acc[:, sl],
                                        in1=n2c[:], op=mybir.AluOpType.mult)
                hcc = work.tile([CLS, 896], dt.bfloat16, tag="hcc")
                nc.sync.dma_start(out=hcc[:], in_=h01n_dram[:, sl])
                nc.vector.tensor_tensor(out=acc[:, sl], in0=acc[:, sl],
                                        in1=hcc[:], op=mybir.AluOpType.add)
                nc.vector.tensor_copy(rho[:, sl], acc[:, sl])

        # ---------------- softmax ----------------
        for ch in range(0, SHPAD, 896):
            sl = slice(ch, ch + 896)
            ric = work.tile([CLS, 896], dt.float32, tag="n2c")
            nc.sync.dma_start(out=ric[:], in_=rinv_d[:, sl])
            nc.vector.tensor_tensor(out=acc[:, sl], in0=rho[:, sl],
                                    in1=ric[:], op=mybir.AluOpType.mult)
        for t in range(NT):
            rT = psum.tile([128, CLS], dt.bfloat16, tag="rT")
            nc.tensor.transpose(out=rT[:],
                                in_=acc[:, t * 128:(t + 1) * 128],
                                identity=ident[:64, :64])
            rt = work.tile([128, CLS], dt.float32, tag="rt")
            nc.scalar.copy(rt[:], rT[:])
            mx = work.tile([128, 1], dt.float32, tag="mx")
            nc.vector.tensor_reduce(mx[:], rt[:], mybir.AxisListType.X,
                                    mybir.AluOpType.max, negate=True)
            ex = work.tile([128, CLS], dt.float32, tag="ex")
            nc.scalar.activation(ex[:], rt[:],
                                 mybir.ActivationFunctionType.Exp,
                                 bias=mx[:])
            sm = work.tile([128, 1], dt.float32, tag="sm")
            nc.vector.tensor_reduce(sm[:], ex[:], mybir.AxisListType.X,
                                    mybir.AluOpType.add)
            rc = work.tile([128, 1], dt.float32, tag="rc")
            nc.vector.reciprocal(rc[:], sm[:])
            ot = work.tile([128, CLS], dt.float32, tag="ot")
            nc.vector.tensor_scalar_mul(ot[:], ex[:], rc[:])
            nc.sync.dma_start(out=out_d[t * 128:(t + 1) * 128, :], in_=ot[:])

    nc.compile()
    return nc


def kernel(features, edge_index, W1, b1, W2, b2):
    features = np.asarray(features, np.float32)
    edge_index = np.asarray(edge_index)
    W1 = np.asarray(W1, np.float32)
    b1 = np.asarray(b1, np.float32)
    W2 = np.asarray(W2, np.float32)
    b2 = np.asarray(b2, np.float32)

    key = (edge_index.shape, int(edge_index[:, :64].sum()),
           int(edge_index[:, -64:].sum()))
    if key not in _cache:
        meta, core_data = _prepare(edge_index)
        nc = _build_program(meta)
        _cache[key] = (nc, meta, core_data)
    nc, meta, core_data = _cache[key]

    in_maps = []
    for c in range(NCORES):
        cd = core_data[c]
        feats = np.zeros((SHPAD, F), np.float32)
        feats[:SH] = features[c * SH:(c + 1) * SH]
        norm = np.zeros(SHPAD, np.float32)
        norm[:SH] = cd["norm"]
        n01 = np.repeat((0.1 * norm)[None, :], CLS, 0).astype(np.float32)
        n2a = np.repeat((ALPHA * norm * norm)[None, :], CLS, 0).astype(
            np.float32)
        rv = np.zeros(SHPAD, np.float32)
        rv[:SH] = 1.0 / cd["norm"]
        rinv = np.repeat(rv[None, :], CLS, 0).astype(np.float32)
        in_maps.append({
            "feats": feats,
            "w1": W1.astype(ml_dtypes.bfloat16),
            "w2": W2.astype(ml_dtypes.bfloat16),
            "b1": b1.reshape(CLS, 1).astype(np.float32),
            "b2": b2.reshape(CLS, 1).astype(np.float32),
            "n01": n01, "n2a": n2a, "rinv": rinv,
            "idxs": cd["idx_blob"], "mgidx": cd["mg_blob"],
        })
    res = run_bass_kernel_spmd(nc, in_maps, core_ids=list(range(NCORES)))
    out = np.empty((N, CLS), np.float32)
    for c in range(NCORES):
        out[c * SH:(c + 1) * SH] = \
            np.asarray(res.results[c]["out"])[:SH].astype(np.float32)
    return out
